# revision 30
# baseline (speedup 1.0000x reference)
"""MiniBatchDiscrimination Trainium2 kernel (8-core SPMD, circulant strips).

Reference computation:
    m = (x @ T).reshape(B, OUT_F, NUM_K)            # B=256, OUT_F=128, NUM_K=16
    dists = |m[None,:,:,:] - m[:,None,:,:]|         # [B, B, OUT_F, NUM_K]
    out = sum_i exp(-sum_k dists) - 1               # [B, OUT_F]
    return concat([x, out], axis=-1)                # [B, 640]

Strategy (identical SPMD program; per-core data = a column permutation):
  * The BxB pair matrix is covered once per unordered pair by 16 "strips"
    plus 8 "diagonal tiles":
      strip a    = {i in 16-block a} x {j in 16-blocks a..a+7 (mod 16)};
      diag tile  = {i in block b} x {j in block b+8}, b = 0..7.
    A strip contributes row-sums for its j's (partial_b, sum over i) and,
    for window blocks a+1..a+7, row-sums for its i's (partial_a, sum over
    j).  Diag tiles contribute both partials.  Exact cover: source-block
    offset e=(bj-bi)%16 is counted by strip-partial_b iff e in {0}u[9,15],
    by strip-partial_a iff e in [1,7], by a diag tile iff e == 8.  Host
    accumulates partials from all cores and subtracts 1.
  * Core c owns strips 2c, 2c+1 and diag tile c.  The window union is 9
    consecutive 16-blocks (144 cols) + blocks c and c+8 (32 cols) -> 176
    "virtual" columns; the host permutes x's rows per core so the program
    is core-independent.
  * Per core: GEMM -> m2T[p=(f8,k), fo, vcol] (contiguous PSUM->SBUF
    casts) -> DVE rearrange to m2[p, vcol, fo] (fo innermost so the
    pairwise subs run in DVE 2x mode).  Then per (strip, 32-j chunk) and
    the diag tile: tensor_sub (DVE 2x) -> |.| (split ACT Abs / DVE 4x
    sign-strip) -> k-sum on TensorE (block-diagonal ones, FD=512) -> Exp
    on ACT -> partial_b ones-matmul into packed PSUM stripes (drained as
    each bank completes); partial_a first-level pair-sum on DVE into SBUF
    slots (host finishes the j-reduction).
"""

import os
import numpy as np

import concourse.bass as bass
import concourse.tile as tile
from concourse import bacc, mybir

BF16 = mybir.dt.bfloat16
FP32 = mybir.dt.float32
U16 = mybir.dt.uint16
NPBF16 = np.dtype(mybir.dt.np(BF16))

B = 256
IN_F = 512
OUT_F = 128
NUM_K = 16
N_CORES = 8
F8 = 8
FO = OUT_F // F8           # 16 fo groups (free dim)
KC = IN_F // 128           # 4 contraction chunks
VCOLS = 176                # 9-block window union + diag i/j blocks
NST = 2                    # strips per core
CH8 = [(0, 32), (32, 32), (64, 32), (96, 32)]  # (window j-off, len)

# abs engine per unit (strip-major, chunk, i-half; last 2 = diag halves):
# A=ACT, D=DVE.
_DEF = "AADAADAD" "AADAADAD" "DD"
ABS_SCHED = os.environ.get("ABS_SCHED", _DEF)


def build_nc():
    nc = bacc.Bacc(name="mbd_strips")

    xT_d = nc.dram_tensor("xT", [128, KC, VCOLS], BF16, kind="ExternalInput")
    T_d = nc.dram_tensor("T_w", [FO, 128, KC, 128], BF16, kind="ExternalInput")
    onk_d = nc.dram_tensor("ones_k", [128, 8 * 64], BF16, kind="ExternalInput")
    ona_d = nc.dram_tensor("ones_acc", [128, F8], BF16, kind="ExternalInput")
    accB_d = nc.dram_tensor("accB", [128, 3, 512], FP32, kind="ExternalOutput")
    accA_d = nc.dram_tensor("accA", [128, 9, 16, FO], BF16,
                            kind="ExternalOutput")

    with tile.TileContext(nc) as tc:
        with (
            tc.tile_pool(name="const", bufs=1) as constp,
            tc.tile_pool(name="gpsum", bufs=2, space=bass.MemorySpace.PSUM) as gps,
            tc.tile_pool(name="dpsum", bufs=3, space=bass.MemorySpace.PSUM) as dps,
            tc.tile_pool(name="bpsum", bufs=1, space=bass.MemorySpace.PSUM) as bps,
            tc.tile_pool(name="diffp", bufs=3) as dfp,
            tc.tile_pool(name="adp", bufs=3) as adp,
            tc.tile_pool(name="expp", bufs=3) as ep,
            tc.tile_pool(name="outp", bufs=1) as op_,
        ):
            # ---- inputs (xT first: GEMM-critical) ----
            xT_sb = constp.tile([128, KC, VCOLS], BF16)
            nc.sync.dma_start(xT_sb[:], xT_d[:])
            T_tiles = []
            dma_engs = [nc.scalar, nc.gpsimd, nc.sync]
            for fo in range(FO):
                tt = constp.tile([128, KC, 128], BF16, tag=f"T{fo}")
                dma_engs[fo % 3].dma_start(tt[:], T_d[fo])
                T_tiles.append(tt)
            ones_k = constp.tile([128, 8, 64], BF16)
            nc.scalar.dma_start(ones_k[:], onk_d.rearrange("p (s q) -> p s q", q=64))
            ones_a = constp.tile([128, F8], BF16)
            nc.gpsimd.dma_start(ones_a[:], ona_d[:])

            zero_b = constp.tile([128, 1], FP32)
            nc.gpsimd.memset(zero_b[:], 0.0)
            # warm the ACT exp/abs tables while DMAs land
            warm = constp.tile([128, 1], FP32)
            nc.scalar.activation(
                warm[:], zero_b[:], mybir.ActivationFunctionType.Exp, bias=zero_b[:]
            )
            # warm the PE p-state with dummy matmuls while DMAs land
            wsb = constp.tile([128, 512], BF16)
            nc.vector.memset(wsb[:], 0.0)
            for w in range(14):
                wpm = gps.tile([128, 512], FP32, tag="gemm", name=f"wpm{w}")
                nc.tensor.matmul(wpm[:], wsb[:, :128], wsb[:], start=True,
                                 stop=True)

            # ---- GEMM: m2T[p, fo, vcol] -> rearrange -> m2[p, vcol, fo] ----
            m2T = constp.tile([128, FO, VCOLS], BF16)
            m2 = constp.tile([128, VCOLS, FO], BF16)
            for fo in range(FO):
                pm = gps.tile([128, VCOLS], FP32, tag="gemm")
                for c in range(KC):
                    nc.tensor.matmul(
                        pm[:],
                        T_tiles[fo][:, c, :],
                        xT_sb[:, c, :],
                        start=(c == 0),
                        stop=(c == KC - 1),
                    )
                nc.vector.tensor_copy(m2T[:, fo, :], pm[:])
            nc.vector.tensor_copy(
                m2[:, 0:44, :], m2T[:, :, 0:44].rearrange("p f v -> p v f")
            )
            for r in range(1, 4):
                vr = slice(44 * r, 44 * r + 44)
                nc.scalar.copy(
                    m2[:, vr, :], m2T[:, :, vr].rearrange("p f v -> p v f")
                )

            # ---- persistent accumulators ----
            accB = []
            for b in range(3):
                accB_t = bps.tile([128, 512], FP32, tag=f"accB{b}", name=f"accB{b}")
                accB.append(accB_t)
            accA_sb = op_.tile([128, 9, 16, FO], BF16)
            fin = op_.tile([128, 3, 512], FP32)

            # units: (i-vcol, j-vcol, jn, pa_lo, pa_hi, stripe idx, accA slot)
            units = []
            for st in range(NST):
                for ci, (joff, jn) in enumerate(CH8):
                    lo = max(16 - joff, 0)
                    hi = min(128 - joff, jn)
                    units.append((st * 16, st * 16 + joff, jn, lo, hi,
                                  st * 4 + ci, st * 4 + ci))
            units.append((144, 160, 16, 0, 16, 8, 8))  # diag tile

            u = 0
            for (ivc, jvc, jn, lo, hi, sidx, slot) in units:
                fd = jn * FO
                pd = dps.tile([128, 512], FP32, tag="dist")
                for h in range(2):
                    i0 = ivc + h * 8
                    diff = dfp.tile([128, 8, 32, FO], BF16, tag="diff")
                    nc.vector.tensor_sub(
                        diff[:, :, :jn, :],
                        m2[:, i0:i0 + 8, None, :].broadcast_to(
                            [128, 8, jn, FO]
                        ),
                        m2[:, None, jvc:jvc + jn, :].broadcast_to(
                            [128, 8, jn, FO]
                        ),
                    )
                    ad = adp.tile([128, 8, 32, FO], BF16, tag="absd")
                    if ABS_SCHED[u] == "A":
                        nc.scalar.activation(
                            ad[:, :, :jn, :], diff[:, :, :jn, :],
                            mybir.ActivationFunctionType.Abs, bias=zero_b[:],
                        )
                    else:
                        nc.vector.tensor_scalar(
                            ad[:, :, :jn, :].bitcast(U16),
                            diff[:, :, :jn, :].bitcast(U16),
                            0x7FFF, None, op0=mybir.AluOpType.bitwise_and,
                        )
                    u += 1
                    for q in range(8):
                        nc.tensor.matmul(
                            pd[h * 64:(h + 1) * 64, :fd],
                            ones_k[:, q, :],
                            ad[:, q, :jn, :],
                            start=(q == 0),
                            stop=(q == 7),
                        )
                et = ep.tile([128, 512], BF16, tag="expt")
                nc.scalar.activation(
                    et[:, :fd], pd[:, :fd],
                    mybir.ActivationFunctionType.Exp, bias=zero_b[:], scale=-1.0,
                )
                # partial_b -> packed psum stripe
                poff = 32 * (sidx % 3)
                nc.tensor.matmul(
                    accB[sidx // 3][poff:poff + 8, :fd],
                    ones_a[:],
                    et[:, :fd],
                    start=True,
                    stop=True,
                    skip_group_check=True,
                )
                # drain a bank as soon as its last stripe lands
                if sidx % 3 == 2:
                    b = sidx // 3
                    nc.scalar.copy(fin[:, b, :], accB[b][:])
                    nc.sync.dma_start(accB_d[:, b], fin[:, b, :])
                # partial_a: single pair-sum level into SBUF (host finishes)
                if lo < hi:
                    e = hi - lo
                    half = e // 2
                    nc.vector.tensor_add(
                        accA_sb[:, slot, :half, :],
                        et[:, lo * FO:(lo + half) * FO],
                        et[:, (lo + half) * FO:(lo + e) * FO],
                    )
                if sidx == 3:
                    nc.gpsimd.dma_start(accA_d[:, 0:4], accA_sb[:, 0:4])
                elif sidx == 7:
                    nc.gpsimd.dma_start(accA_d[:, 4:8], accA_sb[:, 4:8])
                elif sidx == 8:
                    nc.gpsimd.dma_start(accA_d[:, 8:9], accA_sb[:, 8:9])

    nc.finalize()
    return nc


def _vcol_real(c):
    """virtual column -> real row index, for core c."""
    vb = np.arange(144) // 16
    s = np.arange(144) % 16
    return np.concatenate([
        ((2 * c + vb) % 16) * 16 + s,
        (c % 16) * 16 + np.arange(16),
        ((c + 8) % 16) * 16 + np.arange(16),
    ])


def _units():
    units = []
    for st in range(NST):
        for ci, (joff, jn) in enumerate(CH8):
            lo = max(16 - joff, 0)
            hi = min(128 - joff, jn)
            units.append((st * 16, st * 16 + joff, jn, lo, hi,
                          st * 4 + ci, st * 4 + ci))
    units.append((144, 160, 16, 0, 16, 8, 8))
    return units


def make_in_maps(x: np.ndarray, T: np.ndarray):
    # xT_h[p, ch, i] = x[i, ch*128+p]
    xT_h = np.ascontiguousarray(
        x.T.astype(NPBF16).reshape(KC, 128, B).transpose(1, 0, 2)
    )
    T_b = np.ascontiguousarray(T).astype(NPBF16)  # [512, 2048]

    p = np.arange(128)[:, None]
    r = np.arange(F8)[None, :]
    ones_a = np.ascontiguousarray((p % 8 == r).astype(NPBF16))  # [128,8]
    q = np.arange(64)[None, None, :]
    s = np.arange(8)[None, :, None]
    ones_k = (q == s * 8 + p[:, :, None] // 16).astype(NPBF16)
    ones_k = np.ascontiguousarray(ones_k.reshape(128, 8 * 64))

    # T_w[fo, p, c, n] = T[c*128+p, fo*128+n]
    T_perm = np.ascontiguousarray(
        T_b.reshape(KC, 128, FO, 128).transpose(2, 1, 0, 3)
    )

    in_maps = []
    for c in range(N_CORES):
        cols = _vcol_real(c)
        in_maps.append({
            "xT": np.ascontiguousarray(xT_h[:, :, cols]),
            "T_w": T_perm,
            "ones_k": ones_k,
            "ones_acc": ones_a,
        })
    return in_maps


def assemble(x: np.ndarray, results) -> np.ndarray:
    out_pair = np.zeros((B, OUT_F), np.float32)
    units = _units()
    for c, res in enumerate(results):
        cols = _vcol_real(c)
        accB = res["accB"].astype(np.float32)        # [128, 3, 512]
        accA = res["accA"].astype(np.float32)        # [128, 9, 16, FO]
        for (ivc, jvc, jn, lo, hi, sidx, slot) in units:
            poff = 32 * (sidx % 3)
            vals = accB[poff:poff + 8, sidx // 3, :jn * FO].reshape(8, jn, FO)
            # out[j, fo*8+f8] += vals[f8, jj, fo]
            rows = cols[jvc:jvc + jn]
            out_pair[rows] += vals.transpose(1, 2, 0).reshape(jn, OUT_F)
            if lo < hi:
                half = (hi - lo) // 2
                pa = accA[:, slot, :half, :].sum(axis=1)   # [128, FO]
                out_pair[cols[ivc:ivc + 16]] += pa.reshape(
                    16, 8, FO).transpose(0, 2, 1).reshape(16, OUT_F)
    out_pair -= 1.0
    out = np.empty((B, IN_F + OUT_F), np.float32)
    out[:, :IN_F] = x
    out[:, IN_F:] = out_pair
    return out


_NC_CACHE = None


def kernel(x: np.ndarray, T: np.ndarray) -> np.ndarray:
    global _NC_CACHE
    from concourse import bass_utils

    if _NC_CACHE is None:
        _NC_CACHE = build_nc()
    nc = _NC_CACHE
    in_maps = make_in_maps(np.asarray(x, np.float32), np.asarray(T, np.float32))
    res = bass_utils.run_bass_kernel_spmd(nc, in_maps, core_ids=list(range(N_CORES)))
    return assemble(np.asarray(x, np.float32), res.results)


# revision 31
# speedup vs baseline: 1.0080x; 1.0080x over previous
"""MiniBatchDiscrimination Trainium2 kernel (8-core SPMD, circulant strips).

Reference computation:
    m = (x @ T).reshape(B, OUT_F, NUM_K)            # B=256, OUT_F=128, NUM_K=16
    dists = |m[None,:,:,:] - m[:,None,:,:]|         # [B, B, OUT_F, NUM_K]
    out = sum_i exp(-sum_k dists) - 1               # [B, OUT_F]
    return concat([x, out], axis=-1)                # [B, 640]

Strategy (identical SPMD program; per-core data = a column permutation):
  * The BxB pair matrix is covered once per unordered pair by 16 "strips"
    plus 8 "diagonal tiles":
      strip a    = {i in 16-block a} x {j in 16-blocks a..a+7 (mod 16)};
      diag tile  = {i in block b} x {j in block b+8}, b = 0..7.
    A strip contributes row-sums for its j's (partial_b, sum over i) and,
    for window blocks a+1..a+7, row-sums for its i's (partial_a, sum over
    j).  Diag tiles contribute both partials.  Exact cover: source-block
    offset e=(bj-bi)%16 is counted by strip-partial_b iff e in {0}u[9,15],
    by strip-partial_a iff e in [1,7], by a diag tile iff e == 8.  Host
    accumulates partials from all cores and subtracts 1.
  * Core c owns strips 2c, 2c+1 and diag tile c.  The window union is 9
    consecutive 16-blocks (144 cols) + blocks c and c+8 (32 cols) -> 176
    "virtual" columns; the host permutes x's rows per core so the program
    is core-independent.
  * Per core: GEMM -> m2T[p=(f8,k), fo, vcol] (contiguous PSUM->SBUF
    casts) -> DVE rearrange to m2[p, vcol, fo] (fo innermost so the
    pairwise subs run in DVE 2x mode).  Then per (strip, 32-j chunk) and
    the diag tile: tensor_sub (DVE 2x) -> |.| (split ACT Abs / DVE 4x
    sign-strip) -> k-sum on TensorE (block-diagonal ones, FD=512) -> Exp
    on ACT -> partial_b ones-matmul into packed PSUM stripes (drained as
    each bank completes); partial_a first-level pair-sum on DVE into SBUF
    slots (host finishes the j-reduction).
"""

import os
import numpy as np

import concourse.bass as bass
import concourse.tile as tile
from concourse import bacc, mybir

BF16 = mybir.dt.bfloat16
FP32 = mybir.dt.float32
U16 = mybir.dt.uint16
NPBF16 = np.dtype(mybir.dt.np(BF16))

B = 256
IN_F = 512
OUT_F = 128
NUM_K = 16
N_CORES = 8
F8 = 8
FO = OUT_F // F8           # 16 fo groups (free dim)
KC = IN_F // 128           # 4 contraction chunks
VCOLS = 176                # 9-block window union + diag i/j blocks
NST = 2                    # strips per core
CH8 = [(0, 32), (32, 32), (64, 32), (96, 32)]  # (window j-off, len)

# abs engine per unit (strip-major, chunk, i-half; last 2 = diag halves):
# A=ACT, D=DVE.
_DEF = "AADAADAD" "AADAADAD" "DD"
ABS_SCHED = os.environ.get("ABS_SCHED", _DEF)


def build_nc():
    nc = bacc.Bacc(name="mbd_strips")

    xT_d = nc.dram_tensor("xT", [128, KC, VCOLS], BF16, kind="ExternalInput")
    T_d = nc.dram_tensor("T_w", [FO, 128, KC, 128], BF16, kind="ExternalInput")
    onk_d = nc.dram_tensor("ones_k", [128, 8 * 64], BF16, kind="ExternalInput")
    ona_d = nc.dram_tensor("ones_acc", [128, F8], BF16, kind="ExternalInput")
    accB_d = nc.dram_tensor("accB", [128, 3, 512], FP32, kind="ExternalOutput")
    accA_d = nc.dram_tensor("accA", [128, 9, 16, FO], BF16,
                            kind="ExternalOutput")

    with tile.TileContext(nc) as tc:
        with (
            tc.tile_pool(name="const", bufs=1) as constp,
            tc.tile_pool(name="gpsum", bufs=2, space=bass.MemorySpace.PSUM) as gps,
            tc.tile_pool(name="dpsum", bufs=3, space=bass.MemorySpace.PSUM) as dps,
            tc.tile_pool(name="bpsum", bufs=1, space=bass.MemorySpace.PSUM) as bps,
            tc.tile_pool(name="diffp", bufs=3) as dfp,
            tc.tile_pool(name="adp", bufs=3) as adp,
            tc.tile_pool(name="expp", bufs=3) as ep,
            tc.tile_pool(name="outp", bufs=1) as op_,
        ):
            # ---- inputs (xT first: GEMM-critical) ----
            xT_sb = constp.tile([128, KC, VCOLS], BF16)
            nc.sync.dma_start(xT_sb[:], xT_d[:])
            T_tiles = []
            dma_engs = [nc.scalar, nc.gpsimd, nc.sync]
            for fo in range(FO):
                tt = constp.tile([128, KC, 128], BF16, tag=f"T{fo}")
                dma_engs[fo % 3].dma_start(tt[:], T_d[fo])
                T_tiles.append(tt)
            ones_k = constp.tile([128, 8, 64], BF16)
            nc.scalar.dma_start(ones_k[:], onk_d.rearrange("p (s q) -> p s q", q=64))
            ones_a = constp.tile([128, F8], BF16)
            nc.gpsimd.dma_start(ones_a[:], ona_d[:])

            zero_b = constp.tile([128, 1], FP32)
            nc.gpsimd.memset(zero_b[:], 0.0)
            # warm the ACT exp/abs tables while DMAs land
            warm = constp.tile([128, 1], FP32)
            nc.scalar.activation(
                warm[:], zero_b[:], mybir.ActivationFunctionType.Exp, bias=zero_b[:]
            )
            # warm the PE p-state with dummy matmuls while DMAs land
            wsb = constp.tile([128, 512], BF16)
            nc.vector.memset(wsb[:], 0.0)
            for w in range(14):
                wpm = gps.tile([128, 512], FP32, tag="gemm", name=f"wpm{w}")
                nc.tensor.matmul(wpm[:], wsb[:, :128], wsb[:], start=True,
                                 stop=True)

            # ---- GEMM: m2T[p, fo, vcol] -> rearrange -> m2[p, vcol, fo] ----
            m2T = constp.tile([128, FO, VCOLS], BF16)
            m2 = constp.tile([128, VCOLS, FO], BF16)
            for fo in range(FO):
                pm = gps.tile([128, VCOLS], FP32, tag="gemm")
                for c in range(KC):
                    nc.tensor.matmul(
                        pm[:],
                        T_tiles[fo][:, c, :],
                        xT_sb[:, c, :],
                        start=(c == 0),
                        stop=(c == KC - 1),
                    )
                nc.vector.tensor_copy(m2T[:, fo, :], pm[:])
            for r in range(4):
                vr = slice(44 * r, 44 * r + 44)
                nc.vector.tensor_copy(
                    m2[:, vr, :], m2T[:, :, vr].rearrange("p f v -> p v f")
                )

            # ---- persistent accumulators ----
            accB = []
            for b in range(3):
                accB_t = bps.tile([128, 512], FP32, tag=f"accB{b}", name=f"accB{b}")
                accB.append(accB_t)
            accA_sb = op_.tile([128, 9, 16, FO], BF16)
            fin = op_.tile([128, 3, 512], FP32)

            # units: (i-vcol, j-vcol, jn, pa_lo, pa_hi, stripe idx, accA slot)
            units = []
            for st in range(NST):
                for ci, (joff, jn) in enumerate(CH8):
                    lo = max(16 - joff, 0)
                    hi = min(128 - joff, jn)
                    units.append((st * 16, st * 16 + joff, jn, lo, hi,
                                  st * 4 + ci, st * 4 + ci))
            units.append((144, 160, 16, 0, 16, 8, 8))  # diag tile

            u = 0
            for (ivc, jvc, jn, lo, hi, sidx, slot) in units:
                fd = jn * FO
                pd = dps.tile([128, 512], FP32, tag="dist")
                for h in range(2):
                    i0 = ivc + h * 8
                    diff = dfp.tile([128, 8, 32, FO], BF16, tag="diff")
                    nc.vector.tensor_sub(
                        diff[:, :, :jn, :],
                        m2[:, i0:i0 + 8, None, :].broadcast_to(
                            [128, 8, jn, FO]
                        ),
                        m2[:, None, jvc:jvc + jn, :].broadcast_to(
                            [128, 8, jn, FO]
                        ),
                    )
                    ad = adp.tile([128, 8, 32, FO], BF16, tag="absd")
                    if ABS_SCHED[u] == "A":
                        nc.scalar.activation(
                            ad[:, :, :jn, :], diff[:, :, :jn, :],
                            mybir.ActivationFunctionType.Abs, bias=zero_b[:],
                        )
                    else:
                        nc.vector.tensor_scalar(
                            ad[:, :, :jn, :].bitcast(U16),
                            diff[:, :, :jn, :].bitcast(U16),
                            0x7FFF, None, op0=mybir.AluOpType.bitwise_and,
                        )
                    u += 1
                    for q in range(8):
                        nc.tensor.matmul(
                            pd[h * 64:(h + 1) * 64, :fd],
                            ones_k[:, q, :],
                            ad[:, q, :jn, :],
                            start=(q == 0),
                            stop=(q == 7),
                        )
                et = ep.tile([128, 512], BF16, tag="expt")
                nc.scalar.activation(
                    et[:, :fd], pd[:, :fd],
                    mybir.ActivationFunctionType.Exp, bias=zero_b[:], scale=-1.0,
                )
                # partial_b -> packed psum stripe
                poff = 32 * (sidx % 3)
                nc.tensor.matmul(
                    accB[sidx // 3][poff:poff + 8, :fd],
                    ones_a[:],
                    et[:, :fd],
                    start=True,
                    stop=True,
                    skip_group_check=True,
                )
                # drain a bank as soon as its last stripe lands
                if sidx % 3 == 2:
                    b = sidx // 3
                    nc.scalar.copy(fin[:, b, :], accB[b][:])
                    nc.sync.dma_start(accB_d[:, b], fin[:, b, :])
                # partial_a: single pair-sum level into SBUF (host finishes)
                if lo < hi:
                    e = hi - lo
                    half = e // 2
                    nc.vector.tensor_add(
                        accA_sb[:, slot, :half, :],
                        et[:, lo * FO:(lo + half) * FO],
                        et[:, (lo + half) * FO:(lo + e) * FO],
                    )
                if sidx == 3:
                    nc.gpsimd.dma_start(accA_d[:, 0:4], accA_sb[:, 0:4])
                elif sidx == 7:
                    nc.gpsimd.dma_start(accA_d[:, 4:8], accA_sb[:, 4:8])
                elif sidx == 8:
                    nc.gpsimd.dma_start(accA_d[:, 8:9], accA_sb[:, 8:9])

    nc.finalize()
    return nc


def _vcol_real(c):
    """virtual column -> real row index, for core c."""
    vb = np.arange(144) // 16
    s = np.arange(144) % 16
    return np.concatenate([
        ((2 * c + vb) % 16) * 16 + s,
        (c % 16) * 16 + np.arange(16),
        ((c + 8) % 16) * 16 + np.arange(16),
    ])


def _units():
    units = []
    for st in range(NST):
        for ci, (joff, jn) in enumerate(CH8):
            lo = max(16 - joff, 0)
            hi = min(128 - joff, jn)
            units.append((st * 16, st * 16 + joff, jn, lo, hi,
                          st * 4 + ci, st * 4 + ci))
    units.append((144, 160, 16, 0, 16, 8, 8))
    return units


def make_in_maps(x: np.ndarray, T: np.ndarray):
    # xT_h[p, ch, i] = x[i, ch*128+p]
    xT_h = np.ascontiguousarray(
        x.T.astype(NPBF16).reshape(KC, 128, B).transpose(1, 0, 2)
    )
    T_b = np.ascontiguousarray(T).astype(NPBF16)  # [512, 2048]

    p = np.arange(128)[:, None]
    r = np.arange(F8)[None, :]
    ones_a = np.ascontiguousarray((p % 8 == r).astype(NPBF16))  # [128,8]
    q = np.arange(64)[None, None, :]
    s = np.arange(8)[None, :, None]
    ones_k = (q == s * 8 + p[:, :, None] // 16).astype(NPBF16)
    ones_k = np.ascontiguousarray(ones_k.reshape(128, 8 * 64))

    # T_w[fo, p, c, n] = T[c*128+p, fo*128+n]
    T_perm = np.ascontiguousarray(
        T_b.reshape(KC, 128, FO, 128).transpose(2, 1, 0, 3)
    )

    in_maps = []
    for c in range(N_CORES):
        cols = _vcol_real(c)
        in_maps.append({
            "xT": np.ascontiguousarray(xT_h[:, :, cols]),
            "T_w": T_perm,
            "ones_k": ones_k,
            "ones_acc": ones_a,
        })
    return in_maps


def assemble(x: np.ndarray, results) -> np.ndarray:
    out_pair = np.zeros((B, OUT_F), np.float32)
    units = _units()
    for c, res in enumerate(results):
        cols = _vcol_real(c)
        accB = res["accB"].astype(np.float32)        # [128, 3, 512]
        accA = res["accA"].astype(np.float32)        # [128, 9, 16, FO]
        for (ivc, jvc, jn, lo, hi, sidx, slot) in units:
            poff = 32 * (sidx % 3)
            vals = accB[poff:poff + 8, sidx // 3, :jn * FO].reshape(8, jn, FO)
            # out[j, fo*8+f8] += vals[f8, jj, fo]
            rows = cols[jvc:jvc + jn]
            out_pair[rows] += vals.transpose(1, 2, 0).reshape(jn, OUT_F)
            if lo < hi:
                half = (hi - lo) // 2
                pa = accA[:, slot, :half, :].sum(axis=1)   # [128, FO]
                out_pair[cols[ivc:ivc + 16]] += pa.reshape(
                    16, 8, FO).transpose(0, 2, 1).reshape(16, OUT_F)
    out_pair -= 1.0
    out = np.empty((B, IN_F + OUT_F), np.float32)
    out[:, :IN_F] = x
    out[:, IN_F:] = out_pair
    return out


_NC_CACHE = None


def kernel(x: np.ndarray, T: np.ndarray) -> np.ndarray:
    global _NC_CACHE
    from concourse import bass_utils

    if _NC_CACHE is None:
        _NC_CACHE = build_nc()
    nc = _NC_CACHE
    in_maps = make_in_maps(np.asarray(x, np.float32), np.asarray(T, np.float32))
    res = bass_utils.run_bass_kernel_spmd(nc, in_maps, core_ids=list(range(N_CORES)))
    return assemble(np.asarray(x, np.float32), res.results)
